# revision 6
# baseline (speedup 1.0000x reference)
"""Trainium2 Bass kernel for nn_Attention_36644660969693.

Multi-head attention block: x[8,32,32,768] -> qkv -> 12-head attention -> wo.
Sharding: data-parallel over batch, one image (1024 tokens) per NeuronCore.

Per-core layout strategy (T=1024 tokens, C=768, 12 heads, hd=64):
  - xT[c,t] via PE transpose of x
  - qkT[f,t] = w_qkv[:, :1536].T-tile-stationary @ xT  (q,k transposed; head h
    lands at partition (h*64)%128 of f-tile h//2, so a head PAIR occupies the
    two partition halves of one tile -> 2-head row-packed score matmuls)
  - v[t,f] natural orientation, stored padded with interleaved ones columns:
    even head [v|1], odd head [1|v]; the AV matmul then produces the softmax
    row-sums in the complementary 64 output partitions for free
  - scoresT[j,i] per head = kT-tile-stationary @ qT (K=64, two heads packed
    into disjoint PE row groups), exp on ScalarE reads PSUM directly with the
    1/8 scale fused, output fp16 to SBUF
  - AV accumulates v_pad.T @ expT over key tiles; normalize = reciprocal of
    the row-sum half + 64-partition swap DMA + elementwise multiply
  - out[t,:] = attn_T-tile-stationary @ w_o  (natural layout, DMA straight out)
All matmuls run in fp16 (1 cycle/row on PE) with fp32 PSUM accumulation.
"""

import numpy as np

import concourse.bass as bass
import concourse.tile as tile
from concourse import bacc, mybir
from concourse import bass_utils
from concourse import masks

P = 128          # partitions
T = 1024         # tokens per image
C = 768          # model dim
NT = T // P      # 8 token tiles
NC = C // P      # 6 channel tiles
NH = 12          # heads
HD = 64          # head dim
NPAIR = NH // 2  # 6 head pairs
SCALE = HD ** -0.5
F32 = mybir.dt.float32
F16 = mybir.dt.float16
EXP = mybir.ActivationFunctionType.Exp


def _emit_qk_tile(nc, pools, ft):
    """qkT[f-tile ft][:, :] = sum_ct w_qkv[ct, ft*128:+128].T @ xT[ct]."""
    pp_mm, wq, xT, qkT = pools["pp_mm"], pools["wq"], pools["xT"], pools["qkT"]
    for ch in range(2):
        ps = pp_mm.tile([P, 512], F32, tag="mm")
        for ct in range(NC):
            nc.tensor.matmul(
                ps[:],
                wq[:, ct * 2304 + ft * P: ct * 2304 + ft * P + P],
                xT[:, ct * T + ch * 512: ct * T + ch * 512 + 512],
                start=(ct == 0),
                stop=(ct == NC - 1),
            )
        nc.vector.tensor_copy(qkT[:, ft * T + ch * 512: ft * T + ch * 512 + 512], ps[:])


def attention_kernel(tc, out_d, x_d, wq_d, wo_d):
    nc = tc.nc
    from contextlib import ExitStack

    with ExitStack() as ctx:
        const_pool = ctx.enter_context(tc.tile_pool(name="const", bufs=1))
        persist = ctx.enter_context(tc.tile_pool(name="persist", bufs=1))
        opool = ctx.enter_context(tc.tile_pool(name="ot", bufs=2))
        stage_wo = ctx.enter_context(tc.tile_pool(name="swo", bufs=2))
        pp_mm = ctx.enter_context(tc.tile_pool(name="ppmm", bufs=2, space="PSUM"))

        ident = const_pool.tile([P, P], F32, tag="ident")
        masks.make_identity(nc, ident[:])

        xT = persist.tile([P, NC * T], F16, tag="xT")        # [c, t] blocks
        wq = persist.tile([P, NC * 2304], F16, tag="wq")     # [c, f] blocks
        qkT = persist.tile([P, 12 * T], F16, tag="qkT")      # [f, t] blocks
        vpad = persist.tile([P, NT * 1536], F16, tag="vpad")  # [t, padded f]
        aT = persist.tile([P, NC * T], F16, tag="aT")        # [c, t] blocks
        wo_sb = persist.tile([P, NC * C], F16, tag="wo")     # [c, c'] blocks

        pools = {"pp_mm": pp_mm, "wq": wq, "xT": xT, "qkT": qkT}

        # ones blocks of v_pad: per pair block of 256 cols, cols 64:192 are
        # the even head's trailing ones + odd head's leading ones
        ones_ap = vpad[:].rearrange(
            "p (tt pr blk) -> p tt pr blk", tt=NT, pr=6
        )[:, :, :, 64:192]
        nc.gpsimd.memset(ones_ap, 1.0)

        # ---- load + transpose x ----
        with tc.tile_pool(name="sx", bufs=2) as stage_x, \
             tc.tile_pool(name="sw", bufs=2) as stage_w, \
             tc.tile_pool(name="pptr", bufs=2, space="PSUM") as pp_tr:
            for tb in range(2):
                xs = stage_x.tile([P, 4 * C], F32, tag="xs")
                src = x_d[tb * 512:(tb + 1) * 512, :].rearrange(
                    "(k p) c -> p k c", p=P
                )
                nc.gpsimd.dma_start(
                    xs[:].rearrange("p (k c) -> p k c", k=4), src
                )
                for ct in range(NC):
                    ps = pp_tr.tile([P, 512], F32, tag="tr")
                    for k in range(4):
                        nc.tensor.transpose(
                            ps[:, k * P:(k + 1) * P],
                            xs[:, k * C + ct * P: k * C + ct * P + P],
                            ident[:],
                        )
                    dst_off = ct * T + tb * 512
                    nc.vector.tensor_copy(xT[:, dst_off: dst_off + 512], ps[:])

            # ---- load + cast w_qkv ----
            for ct in range(NC):
                ws = stage_w.tile([P, 2304], F32, tag="ws")
                nc.gpsimd.dma_start(ws[:], wq_d[ct * P:(ct + 1) * P, :])
                nc.scalar.copy(wq[:, ct * 2304:(ct + 1) * 2304], ws[:])

            # ---- qk for pair 0 ----
            _emit_qk_tile(nc, pools, 0)
            _emit_qk_tile(nc, pools, 6)

            # ---- v (natural layout, into padded tiles) ----
            for tt in range(NT):
                for (foff, fw) in ((0, 512), (512, 256)):
                    ps = pp_mm.tile([P, 512], F32, tag="mm")
                    for ct in range(NC):
                        nc.tensor.matmul(
                            ps[:, :fw],
                            xT[:, ct * T + tt * P: ct * T + tt * P + P],
                            wq[:, ct * 2304 + 1536 + foff: ct * 2304 + 1536 + foff + fw],
                            start=(ct == 0),
                            stop=(ct == NC - 1),
                        )
                    npr = fw // 128  # head pairs in this chunk
                    src = ps[:, :fw].rearrange("p (m two d) -> p m two d", two=2, d=HD)
                    base = tt * 1536 + (foff // 128) * 256
                    dst = vpad[:, base: base + npr * 256].rearrange(
                        "p (m blk) -> p m blk", blk=256
                    )
                    nc.vector.tensor_copy(dst[:, :, 0:HD], src[:, :, 0, :])
                    nc.vector.tensor_copy(dst[:, :, 192:256], src[:, :, 1, :])

        # ---- load + cast w_o (cast on DVE during attention) ----
        for ct in range(NC):
            ws2 = stage_wo.tile([P, C], F32, tag="ws2")
            nc.gpsimd.dma_start(ws2[:], wo_d[ct * P:(ct + 1) * P, :])
            nc.vector.tensor_copy(wo_sb[:, ct * C:(ct + 1) * C], ws2[:])

        # ---- attention, one head pair at a time ----
        epool = ctx.enter_context(tc.tile_pool(name="E", bufs=3))
        rpool = ctx.enter_context(tc.tile_pool(name="recip", bufs=4))
        pp_s = ctx.enter_context(tc.tile_pool(name="pps", bufs=2, space="PSUM"))
        pp_av = ctx.enter_context(tc.tile_pool(name="ppav", bufs=2, space="PSUM"))

        def normalize(a, h, hp, ch):
            r = rpool.tile([P, 512], F32, tag="r", name=f"r{h}{ch}")
            r2 = rpool.tile([P, 512], F32, tag="r2", name=f"r2{h}{ch}")
            dst = aT[:, hp * T + ch * 512: hp * T + ch * 512 + 512]
            if h == 0:
                nc.vector.reciprocal(r[HD:P, :], a[HD:P, :])
                nc.sync.dma_start(r2[0:HD, :], r[HD:P, :])
                nc.vector.tensor_mul(dst[0:HD, :], a[0:HD, :], r2[0:HD, :])
            else:
                nc.vector.reciprocal(r[0:HD, :], a[0:HD, :])
                nc.sync.dma_start(r2[HD:P, :], r[0:HD, :])
                nc.vector.tensor_mul(dst[HD:P, :], a[HD:P, :], r2[HD:P, :])

        for hp in range(NPAIR):
            qblk = hp * T
            kblk = (6 + hp) * T
            E0 = epool.tile([P, NT * T], F16, tag="E", name="E0")
            E1 = epool.tile([P, NT * T], F16, tag="E", name="E1")
            av00 = pp_av.tile([P, 512], F32, tag="av", name="av00")
            av10 = pp_av.tile([P, 512], F32, tag="av", name="av10")

            qk_next = []
            if hp + 1 < NPAIR:
                qk_next = [hp + 1, 6 + hp + 1]

            for jt in range(NT):
                s0 = pp_s.tile([P, T], F32, tag="s", name="s0")
                s1 = pp_s.tile([P, T], F32, tag="s", name="s1")
                for ch in range(2):
                    nc.tensor.matmul(
                        s0[:, ch * 512:(ch + 1) * 512],
                        qkT[0:HD, kblk + jt * P: kblk + jt * P + P],
                        qkT[0:HD, qblk + ch * 512: qblk + ch * 512 + 512],
                        start=True, stop=True,
                    )
                    nc.tensor.matmul(
                        s1[:, ch * 512:(ch + 1) * 512],
                        qkT[HD:P, kblk + jt * P: kblk + jt * P + P],
                        qkT[HD:P, qblk + ch * 512: qblk + ch * 512 + 512],
                        start=True, stop=True,
                    )
                nc.scalar.activation(E0[:, jt * T:(jt + 1) * T], s0[:], EXP, scale=SCALE)
                nc.scalar.activation(E1[:, jt * T:(jt + 1) * T], s1[:], EXP, scale=SCALE)

                # AV accumulation step (i-chunk 0) for this key tile
                for h, E, a in ((0, E0, av00), (1, E1, av10)):
                    nc.tensor.matmul(
                        a[:],
                        vpad[:, jt * 1536 + (2 * hp + h) * P:
                             jt * 1536 + (2 * hp + h) * P + P],
                        E[:, jt * T: jt * T + 512],
                        start=(jt == 0),
                        stop=(jt == NT - 1),
                    )
                # interleave next pair's qk matmuls to fill PE stalls
                if jt in (1, 3) and qk_next:
                    _emit_qk_tile(nc, pools, qk_next.pop(0))

            normalize(av00, 0, hp, 0)
            normalize(av10, 1, hp, 0)

            # AV pass for i-chunk 1 (all exps for this pair already done)
            av01 = pp_av.tile([P, 512], F32, tag="av", name="av01")
            av11 = pp_av.tile([P, 512], F32, tag="av", name="av11")
            for h, E, a in ((0, E0, av01), (1, E1, av11)):
                for jt in range(NT):
                    nc.tensor.matmul(
                        a[:],
                        vpad[:, jt * 1536 + (2 * hp + h) * P:
                             jt * 1536 + (2 * hp + h) * P + P],
                        E[:, jt * T + 512: jt * T + 1024],
                        start=(jt == 0),
                        stop=(jt == NT - 1),
                    )
            normalize(av01, 0, hp, 1)
            normalize(av11, 1, hp, 1)

        # ---- output projection ----
        for tt in range(NT):
            po1 = pp_mm.tile([P, 512], F32, tag="mm")
            po2 = pp_mm.tile([P, 512], F32, tag="mm")
            for ct in range(NC):
                lhsT = aT[:, ct * T + tt * P: ct * T + tt * P + P]
                nc.tensor.matmul(po1[:], lhsT, wo_sb[:, ct * C: ct * C + 512],
                                 start=(ct == 0), stop=(ct == NC - 1))
            for ct in range(NC):
                lhsT = aT[:, ct * T + tt * P: ct * T + tt * P + P]
                nc.tensor.matmul(po2[:, :256], lhsT, wo_sb[:, ct * C + 512: ct * C + C],
                                 start=(ct == 0), stop=(ct == NC - 1))
            ot = opool.tile([P, C], F32, tag="ot")
            nc.vector.tensor_copy(ot[:, 0:512], po1[:])
            nc.vector.tensor_copy(ot[:, 512:C], po2[:, :256])
            nc.gpsimd.dma_start(out_d[tt * P:(tt + 1) * P, :], ot[:])


_CACHED = {}


def build_program():
    if "nc" in _CACHED:
        return _CACHED["nc"]
    nc = bacc.Bacc("TRN2", target_bir_lowering=False, debug=False, num_devices=8)
    x_d = nc.dram_tensor("x", [T, C], F32, kind="ExternalInput").ap()
    wq_d = nc.dram_tensor("w_qkv", [C, 3 * C], F32, kind="ExternalInput").ap()
    wo_d = nc.dram_tensor("w_o", [C, C], F32, kind="ExternalInput").ap()
    out_d = nc.dram_tensor("out", [T, C], F32, kind="ExternalOutput").ap()
    with tile.TileContext(nc) as tc:
        attention_kernel(tc, out_d, x_d, wq_d, wo_d)
    nc.compile()
    _CACHED["nc"] = nc
    return nc


def kernel(x, w_qkv, w_o, _trace=False, _trace_cores=None):
    nc = build_program()
    x = np.ascontiguousarray(np.asarray(x, dtype=np.float32))
    w_qkv = np.ascontiguousarray(np.asarray(w_qkv, dtype=np.float32))
    w_o = np.ascontiguousarray(np.asarray(w_o, dtype=np.float32))
    bs = x.shape[0]
    in_maps = [
        {"x": x[b].reshape(T, C), "w_qkv": w_qkv, "w_o": w_o} for b in range(bs)
    ]
    res = bass_utils.run_bass_kernel_spmd(
        nc, in_maps, core_ids=list(range(bs)), trace=_trace,
        trace_cores=_trace_cores,
    )
    out = np.stack([res.results[b]["out"].reshape(32, 32, C) for b in range(bs)])
    if _trace:
        return out, res
    return out


# revision 8
# speedup vs baseline: 1.0474x; 1.0474x over previous
"""Trainium2 Bass kernel for nn_Attention_36644660969693.

Multi-head attention block: x[8,32,32,768] -> qkv -> 12-head attention -> wo.
Sharding: data-parallel over batch, one image (1024 tokens) per NeuronCore.

Per-core layout strategy (T=1024 tokens, C=768, 12 heads, hd=64):
  - xT[c,t] via PE transpose of x
  - qkT[f,t] = w_qkv[:, :1536].T-tile-stationary @ xT  (q,k transposed; head h
    lands at partition (h*64)%128 of f-tile h//2, so a head PAIR occupies the
    two partition halves of one tile -> 2-head row-packed score matmuls)
  - v[t,f] natural orientation, stored per pair as [v_even | ones | v_odd]
    (192 cols); the AV stationary [128,128] slice for the even head is
    [v|1] and for the odd head [1|v], so each AV matmul also emits the
    softmax row-sums in the complementary 64 output partitions for free
  - scoresT[j,i] per head = kT-tile-stationary @ qT (K=64, two heads packed
    into disjoint PE row groups), exp on ScalarE reads PSUM directly with the
    1/8 scale fused, output fp16 to SBUF
  - AV accumulates v_pad.T @ expT over key tiles (3 of 4 head/chunk
    accumulations pipelined inside the score/exp loop, lagging exp by one
    tile); normalize = fast-reciprocal of the row-sum half + 64-partition
    swap DMA + elementwise multiply
  - out[t,:] = attn_T-tile-stationary @ w_o  (natural layout, DMA straight out)
All matmuls run in fp16 (1 cycle/row on PE) with fp32 PSUM accumulation.
"""

import numpy as np

import concourse.bass as bass
import concourse.tile as tile
from concourse import bacc, mybir
from concourse import bass_utils
from concourse import masks

P = 128          # partitions
T = 1024         # tokens per image
C = 768          # model dim
NT = T // P      # 8 token tiles
NC = C // P      # 6 channel tiles
NH = 12          # heads
HD = 64          # head dim
NPAIR = NH // 2  # 6 head pairs
VPW = 192        # v_pad pair block width: [v_even(64) | ones(64) | v_odd(64)]
SCALE = HD ** -0.5
F32 = mybir.dt.float32
F16 = mybir.dt.float16
EXP = mybir.ActivationFunctionType.Exp


def _emit_qk_tile(nc, pools, ft):
    """qkT[f-tile ft][:, :] = sum_ct w_qkv[ct, ft*128:+128].T @ xT[ct]."""
    pp, wq, xT, qkT = pools["pp_mm"], pools["wq"], pools["xT"], pools["qkT"]
    for ch in range(2):
        ps = pp.tile([P, 512], F32, tag="mm", name="ps_qk")
        for ct in range(NC):
            nc.tensor.matmul(
                ps[:],
                wq[:, ct * 2304 + ft * P: ct * 2304 + ft * P + P],
                xT[:, ct * T + ch * 512: ct * T + ch * 512 + 512],
                start=(ct == 0),
                stop=(ct == NC - 1),
            )
        nc.vector.tensor_copy(qkT[:, ft * T + ch * 512: ft * T + ch * 512 + 512], ps[:])


def attention_kernel(tc, out_d, x_d, wq_d, wo_d):
    nc = tc.nc
    from contextlib import ExitStack

    with ExitStack() as ctx:
        const_pool = ctx.enter_context(tc.tile_pool(name="const", bufs=1))
        persist = ctx.enter_context(tc.tile_pool(name="persist", bufs=1))
        opool = ctx.enter_context(tc.tile_pool(name="ot", bufs=2))
        stage_wo = ctx.enter_context(tc.tile_pool(name="swo", bufs=2))

        ident = const_pool.tile([P, P], F32, tag="ident")
        masks.make_identity(nc, ident[:])

        xT = persist.tile([P, NC * T], F16, tag="xT")        # [c, t] blocks
        wq = persist.tile([P, NC * 2304], F16, tag="wq")     # [c, f] blocks
        qkT = persist.tile([P, 12 * T], F16, tag="qkT")      # [f, t] blocks
        vpad = persist.tile([P, NT * NPAIR * VPW], F16, tag="vpad")
        aT = persist.tile([P, NC * T], F16, tag="aT")        # [c, t] blocks
        wo_sb = persist.tile([P, NC * C], F16, tag="wo")     # [c, c'] blocks

        pools = {"wq": wq, "xT": xT, "qkT": qkT}

        # ones blocks of v_pad: cols 64:128 of each 192-col pair block
        ones_ap = vpad[:].rearrange(
            "p (blk w) -> p blk w", w=VPW
        )[:, :, HD: 2 * HD]
        nc.gpsimd.memset(ones_ap, 1.0)

        # ---- prep: load + transpose x, load + cast weights, qk pair 0, v ----
        with tc.tile_pool(name="sx", bufs=2) as stage_x, \
             tc.tile_pool(name="sw", bufs=2) as stage_w, \
             tc.tile_pool(name="pprep", bufs=3, space="PSUM") as pp_prep, \
             tc.tile_pool(name="pptr", bufs=2, space="PSUM") as pp_tr:
            pools["pp_mm"] = pp_prep
            for tb in range(2):
                xs = stage_x.tile([P, 4 * C], F32, tag="xs", name="xs")
                src = x_d[tb * 512:(tb + 1) * 512, :].rearrange(
                    "(k p) c -> p k c", p=P
                )
                nc.sync.dma_start(
                    xs[:].rearrange("p (k c) -> p k c", k=4), src
                )
                for ct in range(NC):
                    ps = pp_tr.tile([P, 512], F32, tag="tr", name="ps_tr")
                    for k in range(4):
                        nc.tensor.transpose(
                            ps[:, k * P:(k + 1) * P],
                            xs[:, k * C + ct * P: k * C + ct * P + P],
                            ident[:],
                        )
                    dst_off = ct * T + tb * 512
                    nc.vector.tensor_copy(xT[:, dst_off: dst_off + 512], ps[:])

            for ct in range(NC):
                ws = stage_w.tile([P, 2304], F32, tag="ws", name="ws")
                nc.sync.dma_start(ws[:], wq_d[ct * P:(ct + 1) * P, :])
                nc.scalar.copy(wq[:, ct * 2304:(ct + 1) * 2304], ws[:])

            _emit_qk_tile(nc, pools, 0)
            _emit_qk_tile(nc, pools, 6)

            # ---- v (natural layout, into padded [v|1|v] pair blocks) ----
            for tt in range(NT):
                for (foff, fw) in ((0, 512), (512, 256)):
                    ps = pp_prep.tile([P, 512], F32, tag="mm", name="ps_v")
                    for ct in range(NC):
                        nc.tensor.matmul(
                            ps[:, :fw],
                            xT[:, ct * T + tt * P: ct * T + tt * P + P],
                            wq[:, ct * 2304 + 1536 + foff: ct * 2304 + 1536 + foff + fw],
                            start=(ct == 0),
                            stop=(ct == NC - 1),
                        )
                    npr = fw // 128  # head pairs in this chunk
                    src = ps[:, :fw].rearrange("p (m two d) -> p m two d", two=2, d=HD)
                    base = tt * NPAIR * VPW + (foff // 128) * VPW
                    dst = vpad[:, base: base + npr * VPW].rearrange(
                        "p (m blk) -> p m blk", blk=VPW
                    )
                    nc.vector.tensor_copy(dst[:, :, 0:HD], src[:, :, 0, :])
                    nc.vector.tensor_copy(dst[:, :, 2 * HD:VPW], src[:, :, 1, :])

        # ---- load + cast w_o (cast on GpSimd during attention) ----
        for ct in range(NC):
            ws2 = stage_wo.tile([P, C], F32, tag="ws2", name="ws2")
            nc.sync.dma_start(ws2[:], wo_d[ct * P:(ct + 1) * P, :])
            nc.gpsimd.tensor_copy(wo_sb[:, ct * C:(ct + 1) * C], ws2[:])

        # ---- attention, one head pair at a time ----
        epool = ctx.enter_context(tc.tile_pool(name="E", bufs=3))
        rpool = ctx.enter_context(tc.tile_pool(name="recip", bufs=4))
        pp_mm = ctx.enter_context(tc.tile_pool(name="ppmm", bufs=1, space="PSUM"))
        pp_s = ctx.enter_context(tc.tile_pool(name="pps", bufs=2, space="PSUM"))
        pp_av = ctx.enter_context(tc.tile_pool(name="ppav", bufs=3, space="PSUM"))
        pools["pp_mm"] = pp_mm

        def vslice(jt, hp, h):
            """[128,128] AV stationary: even head [v|1], odd head [1|v]."""
            base = jt * NPAIR * VPW + hp * VPW + (0 if h == 0 else HD)
            return vpad[:, base: base + P]

        def normalize(a, h, hp, ch):
            r = rpool.tile([P, 512], F32, tag="r", name=f"r{h}{ch}")
            r2 = rpool.tile([P, 512], F32, tag="r2", name=f"r2{h}{ch}")
            dst = aT[:, hp * T + ch * 512: hp * T + ch * 512 + 512]
            if h == 0:
                nc.vector.reciprocal(r[HD:P, :], a[HD:P, :])
                nc.sync.dma_start(r2[0:HD, :], r[HD:P, :])
                nc.vector.tensor_mul(dst[0:HD, :], a[0:HD, :], r2[0:HD, :])
            else:
                nc.vector.reciprocal(r[0:HD, :], a[0:HD, :])
                nc.sync.dma_start(r2[HD:P, :], r[0:HD, :])
                nc.vector.tensor_mul(dst[HD:P, :], a[HD:P, :], r2[HD:P, :])

        for hp in range(NPAIR):
            qblk = hp * T
            kblk = (6 + hp) * T
            E0 = epool.tile([P, NT * T], F16, tag="E", name="E0")
            E1 = epool.tile([P, NT * T], F16, tag="E", name="E1")
            av00 = pp_av.tile([P, 512], F32, tag="av", name="av00")
            av10 = pp_av.tile([P, 512], F32, tag="av", name="av10")
            av01 = pp_av.tile([P, 512], F32, tag="av", name="av01")

            def av_steps(jt):
                # 3 of 4 (head, chunk) accumulations pipelined in-loop
                for a, h, E, co in ((av00, 0, E0, 0), (av10, 1, E1, 0),
                                    (av01, 0, E0, 512)):
                    nc.tensor.matmul(
                        a[:],
                        vslice(jt, hp, h),
                        E[:, jt * T + co: jt * T + co + 512],
                        start=(jt == 0),
                        stop=(jt == NT - 1),
                    )

            qk_next = []
            if hp + 1 < NPAIR:
                qk_next = [hp + 1, 6 + hp + 1]

            for jt in range(NT):
                s0 = pp_s.tile([P, T], F32, tag="s", name="s0")
                s1 = pp_s.tile([P, T], F32, tag="s", name="s1")
                for ch in range(2):
                    nc.tensor.matmul(
                        s0[:, ch * 512:(ch + 1) * 512],
                        qkT[0:HD, kblk + jt * P: kblk + jt * P + P],
                        qkT[0:HD, qblk + ch * 512: qblk + ch * 512 + 512],
                        start=True, stop=True,
                    )
                    nc.tensor.matmul(
                        s1[:, ch * 512:(ch + 1) * 512],
                        qkT[HD:P, kblk + jt * P: kblk + jt * P + P],
                        qkT[HD:P, qblk + ch * 512: qblk + ch * 512 + 512],
                        start=True, stop=True,
                    )
                nc.scalar.activation(E0[:, jt * T:(jt + 1) * T], s0[:], EXP, scale=SCALE)
                nc.scalar.activation(E1[:, jt * T:(jt + 1) * T], s1[:], EXP, scale=SCALE)

                if jt > 0:
                    av_steps(jt - 1)   # lag exp by one key tile
                if jt in (1, 3) and qk_next:
                    _emit_qk_tile(nc, pools, qk_next.pop(0))
            av_steps(NT - 1)

            normalize(av00, 0, hp, 0)
            normalize(av10, 1, hp, 0)
            normalize(av01, 0, hp, 1)

            # last accumulation (h1, chunk 1) as a short post-pass
            av11 = pp_av.tile([P, 512], F32, tag="av", name="av11")
            for jt in range(NT):
                nc.tensor.matmul(
                    av11[:],
                    vslice(jt, hp, 1),
                    E1[:, jt * T + 512: jt * T + 1024],
                    start=(jt == 0),
                    stop=(jt == NT - 1),
                )
            normalize(av11, 1, hp, 1)

        # ---- output projection ----
        for tt in range(NT):
            po1 = pp_mm.tile([P, 512], F32, tag="mm", name="po1")
            po2 = pp_av.tile([P, 512], F32, tag="av", name="po2")
            for ct in range(NC):
                lhsT = aT[:, ct * T + tt * P: ct * T + tt * P + P]
                nc.tensor.matmul(po1[:], lhsT, wo_sb[:, ct * C: ct * C + 512],
                                 start=(ct == 0), stop=(ct == NC - 1))
            for ct in range(NC):
                lhsT = aT[:, ct * T + tt * P: ct * T + tt * P + P]
                nc.tensor.matmul(po2[:, :256], lhsT, wo_sb[:, ct * C + 512: ct * C + C],
                                 start=(ct == 0), stop=(ct == NC - 1))
            ot = opool.tile([P, C], F32, tag="ot", name="ot")
            nc.scalar.copy(ot[:, 0:512], po1[:])
            nc.scalar.copy(ot[:, 512:C], po2[:, :256])
            nc.sync.dma_start(out_d[tt * P:(tt + 1) * P, :], ot[:])


_CACHED = {}


def build_program():
    if "nc" in _CACHED:
        return _CACHED["nc"]
    nc = bacc.Bacc("TRN2", target_bir_lowering=False, debug=False, num_devices=8)
    x_d = nc.dram_tensor("x", [T, C], F32, kind="ExternalInput").ap()
    wq_d = nc.dram_tensor("w_qkv", [C, 3 * C], F32, kind="ExternalInput").ap()
    wo_d = nc.dram_tensor("w_o", [C, C], F32, kind="ExternalInput").ap()
    out_d = nc.dram_tensor("out", [T, C], F32, kind="ExternalOutput").ap()
    with tile.TileContext(nc) as tc:
        attention_kernel(tc, out_d, x_d, wq_d, wo_d)
    nc.compile()
    _CACHED["nc"] = nc
    return nc


def kernel(x, w_qkv, w_o, _trace=False, _trace_cores=None):
    nc = build_program()
    x = np.ascontiguousarray(np.asarray(x, dtype=np.float32))
    w_qkv = np.ascontiguousarray(np.asarray(w_qkv, dtype=np.float32))
    w_o = np.ascontiguousarray(np.asarray(w_o, dtype=np.float32))
    bs = x.shape[0]
    in_maps = [
        {"x": x[b].reshape(T, C), "w_qkv": w_qkv, "w_o": w_o} for b in range(bs)
    ]
    res = bass_utils.run_bass_kernel_spmd(
        nc, in_maps, core_ids=list(range(bs)), trace=_trace,
        trace_cores=_trace_cores,
    )
    out = np.stack([res.results[b]["out"].reshape(32, 32, C) for b in range(bs)])
    if _trace:
        return out, res
    return out


# revision 9
# speedup vs baseline: 1.3862x; 1.3234x over previous
"""Trainium2 Bass kernel for nn_Attention_36644660969693.

Multi-head attention block: x[8,32,32,768] -> qkv -> 12-head attention -> wo.
Sharding: data-parallel over batch, one image (1024 tokens) per NeuronCore.

Per-core layout strategy (T=1024 tokens, C=768, 12 heads, hd=64):
  - xT[c,t] via PE transpose of x
  - qkT[f,t] = w_qkv[:, :1536].T-tile-stationary @ xT  (q,k transposed; head h
    lands at partition (h*64)%128 of f-tile h//2, so a head PAIR occupies the
    two partition halves of one tile -> 2-head row-packed score matmuls)
  - v[t,f] natural orientation, stored per pair as [v_even | ones | v_odd]
    (192 cols); the AV stationary [128,128] slice for the even head is
    [v|1] and for the odd head [1|v], so each AV matmul also emits the
    softmax row-sums in the complementary 64 output partitions for free
  - scoresT[j,i] per head = kT-tile-stationary @ qT (K=64, two heads packed
    into disjoint PE row groups), exp on ScalarE reads PSUM directly with the
    1/8 scale fused, output fp16 to SBUF
  - AV accumulates v_pad.T @ expT over key tiles (3 of 4 head/chunk
    accumulations pipelined inside the score/exp loop, lagging exp by one
    tile); normalize = fast-reciprocal of the row-sum half + 64-partition
    swap DMA + elementwise multiply
  - out[t,:] = attn_T-tile-stationary @ w_o  (natural layout, DMA straight out)
All matmuls run in fp16 (1 cycle/row on PE) with fp32 PSUM accumulation.
"""

import numpy as np

import concourse.bass as bass
import concourse.tile as tile
from concourse import bacc, mybir
from concourse import bass_utils
from concourse import masks

P = 128          # partitions
T = 1024         # tokens per image
C = 768          # model dim
NT = T // P      # 8 token tiles
NC = C // P      # 6 channel tiles
NH = 12          # heads
HD = 64          # head dim
NPAIR = NH // 2  # 6 head pairs
VPW = 192        # v_pad pair block width: [v_even(64) | ones(64) | v_odd(64)]
SCALE = HD ** -0.5
F32 = mybir.dt.float32
F16 = mybir.dt.float16
EXP = mybir.ActivationFunctionType.Exp


def _emit_qk_tile(nc, pools, ft):
    """qkT[f-tile ft][:, :] = sum_ct w_qkv[ct, ft*128:+128].T @ xT[ct]."""
    pp, wq, xT, qkT = pools["pp_mm"], pools["wq"], pools["xT"], pools["qkT"]
    for ch in range(2):
        ps = pp.tile([P, 512], F32, tag="mm", name="ps_qk")
        for ct in range(NC):
            nc.tensor.matmul(
                ps[:],
                wq[:, ct * 2304 + ft * P: ct * 2304 + ft * P + P],
                xT[:, ct * T + ch * 512: ct * T + ch * 512 + 512],
                start=(ct == 0),
                stop=(ct == NC - 1),
            )
        nc.vector.tensor_copy(qkT[:, ft * T + ch * 512: ft * T + ch * 512 + 512], ps[:])


def attention_kernel(tc, out_d, x_d, wq_d, wo_d):
    nc = tc.nc
    from contextlib import ExitStack

    with ExitStack() as ctx:
        const_pool = ctx.enter_context(tc.tile_pool(name="const", bufs=1))
        persist = ctx.enter_context(tc.tile_pool(name="persist", bufs=1))
        opool = ctx.enter_context(tc.tile_pool(name="ot", bufs=2))
        stage_wo = ctx.enter_context(tc.tile_pool(name="swo", bufs=2))

        ident = const_pool.tile([P, P], F32, tag="ident")
        masks.make_identity(nc, ident[:])

        xT = persist.tile([P, NC * T], F16, tag="xT")        # [c, t] blocks
        wq = persist.tile([P, NC * 2304], F16, tag="wq")     # [c, f] blocks
        qkT = persist.tile([P, 12 * T], F16, tag="qkT")      # [f, t] blocks
        vpad = persist.tile([P, NT * NPAIR * VPW], F16, tag="vpad")
        aT = persist.tile([P, NC * T], F16, tag="aT")        # [c, t] blocks
        wo_sb = persist.tile([P, NC * C], F16, tag="wo")     # [c, c'] blocks

        pools = {"wq": wq, "xT": xT, "qkT": qkT}

        # ones blocks of v_pad: cols 64:128 of each 192-col pair block
        ones_ap = vpad[:].rearrange(
            "p (blk w) -> p blk w", w=VPW
        )[:, :, HD: 2 * HD]
        nc.gpsimd.memset(ones_ap, 1.0)

        # ---- prep: load + transpose x, load + cast weights, qk pair 0, v ----
        with tc.tile_pool(name="sx", bufs=2) as stage_x, \
             tc.tile_pool(name="sw", bufs=2) as stage_w, \
             tc.tile_pool(name="pprep", bufs=3, space="PSUM") as pp_prep, \
             tc.tile_pool(name="pptr", bufs=2, space="PSUM") as pp_tr:
            pools["pp_mm"] = pp_prep
            for tb in range(2):
                xs = stage_x.tile([P, 4 * C], F32, tag="xs", name="xs")
                src = x_d[tb * 512:(tb + 1) * 512, :].rearrange(
                    "(k p) c -> p k c", p=P
                )
                nc.sync.dma_start(
                    xs[:].rearrange("p (k c) -> p k c", k=4), src
                )
                for ct in range(NC):
                    ps = pp_tr.tile([P, 512], F32, tag="tr", name="ps_tr")
                    for k in range(4):
                        nc.tensor.transpose(
                            ps[:, k * P:(k + 1) * P],
                            xs[:, k * C + ct * P: k * C + ct * P + P],
                            ident[:],
                        )
                    dst_off = ct * T + tb * 512
                    nc.vector.tensor_copy(xT[:, dst_off: dst_off + 512], ps[:])

            for ct in range(NC):
                ws = stage_w.tile([P, 2304], F32, tag="ws", name="ws")
                nc.sync.dma_start(ws[:], wq_d[ct * P:(ct + 1) * P, :])
                nc.scalar.copy(wq[:, ct * 2304:(ct + 1) * 2304], ws[:])

            _emit_qk_tile(nc, pools, 0)
            _emit_qk_tile(nc, pools, 6)

            # ---- v (natural layout, into padded [v|1|v] pair blocks) ----
            for tt in range(NT):
                for (foff, fw) in ((0, 512), (512, 256)):
                    ps = pp_prep.tile([P, 512], F32, tag="mm", name="ps_v")
                    for ct in range(NC):
                        nc.tensor.matmul(
                            ps[:, :fw],
                            xT[:, ct * T + tt * P: ct * T + tt * P + P],
                            wq[:, ct * 2304 + 1536 + foff: ct * 2304 + 1536 + foff + fw],
                            start=(ct == 0),
                            stop=(ct == NC - 1),
                        )
                    npr = fw // 128  # head pairs in this chunk
                    src = ps[:, :fw].rearrange("p (m two d) -> p m two d", two=2, d=HD)
                    base = tt * NPAIR * VPW + (foff // 128) * VPW
                    dst = vpad[:, base: base + npr * VPW].rearrange(
                        "p (m blk) -> p m blk", blk=VPW
                    )
                    nc.vector.tensor_copy(dst[:, :, 0:HD], src[:, :, 0, :])
                    nc.vector.tensor_copy(dst[:, :, 2 * HD:VPW], src[:, :, 1, :])

        # ---- load + cast w_o (cast on GpSimd during attention) ----
        for ct in range(NC):
            ws2 = stage_wo.tile([P, C], F32, tag="ws2", name="ws2")
            nc.sync.dma_start(ws2[:], wo_d[ct * P:(ct + 1) * P, :])
            nc.gpsimd.tensor_copy(wo_sb[:, ct * C:(ct + 1) * C], ws2[:])

        # ---- attention, one head pair at a time ----
        epool = ctx.enter_context(tc.tile_pool(name="E", bufs=3))
        rpool = ctx.enter_context(tc.tile_pool(name="recip", bufs=4))
        pp_mm = ctx.enter_context(tc.tile_pool(name="ppmm", bufs=1, space="PSUM"))
        pp_s = ctx.enter_context(tc.tile_pool(name="pps", bufs=2, space="PSUM"))
        pp_av = ctx.enter_context(tc.tile_pool(name="ppav", bufs=3, space="PSUM"))
        pools["pp_mm"] = pp_mm

        def vslice(jt, hp, h):
            """[128,128] AV stationary: even head [v|1], odd head [1|v]."""
            base = jt * NPAIR * VPW + hp * VPW + (0 if h == 0 else HD)
            return vpad[:, base: base + P]

        def normalize(a, h, hp, ch):
            r = rpool.tile([P, 512], F32, tag="r", name=f"r{h}{ch}")
            r2 = rpool.tile([P, 512], F32, tag="r2", name=f"r2{h}{ch}")
            dst = aT[:, hp * T + ch * 512: hp * T + ch * 512 + 512]
            # full-partition approx reciprocal (custom DVE op needs base
            # partition 0); the non-rowsum half of r is garbage, never read
            nc.vector.reciprocal_approx_fast(r[:, :], a[:, :])
            if h == 0:
                nc.sync.dma_start(r2[0:HD, :], r[HD:P, :])
                nc.vector.tensor_mul(dst[0:HD, :], a[0:HD, :], r2[0:HD, :])
            else:
                nc.sync.dma_start(r2[HD:P, :], r[0:HD, :])
                nc.vector.tensor_mul(dst[HD:P, :], a[HD:P, :], r2[HD:P, :])

        for hp in range(NPAIR):
            qblk = hp * T
            kblk = (6 + hp) * T
            E0 = epool.tile([P, NT * T], F16, tag="E", name="E0")
            E1 = epool.tile([P, NT * T], F16, tag="E", name="E1")
            av00 = pp_av.tile([P, 512], F32, tag="av", name="av00")
            av10 = pp_av.tile([P, 512], F32, tag="av", name="av10")
            av01 = pp_av.tile([P, 512], F32, tag="av", name="av01")

            def av_steps(jt):
                # 3 of 4 (head, chunk) accumulations pipelined in-loop
                for a, h, E, co in ((av00, 0, E0, 0), (av10, 1, E1, 0),
                                    (av01, 0, E0, 512)):
                    nc.tensor.matmul(
                        a[:],
                        vslice(jt, hp, h),
                        E[:, jt * T + co: jt * T + co + 512],
                        start=(jt == 0),
                        stop=(jt == NT - 1),
                    )

            qk_next = []
            if hp + 1 < NPAIR:
                qk_next = [hp + 1, 6 + hp + 1]

            for jt in range(NT):
                s0 = pp_s.tile([P, T], F32, tag="s", name="s0")
                s1 = pp_s.tile([P, T], F32, tag="s", name="s1")
                for ch in range(2):
                    nc.tensor.matmul(
                        s0[:, ch * 512:(ch + 1) * 512],
                        qkT[0:HD, kblk + jt * P: kblk + jt * P + P],
                        qkT[0:HD, qblk + ch * 512: qblk + ch * 512 + 512],
                        start=True, stop=True,
                    )
                    nc.tensor.matmul(
                        s1[:, ch * 512:(ch + 1) * 512],
                        qkT[HD:P, kblk + jt * P: kblk + jt * P + P],
                        qkT[HD:P, qblk + ch * 512: qblk + ch * 512 + 512],
                        start=True, stop=True,
                    )
                nc.scalar.activation(E0[:, jt * T:(jt + 1) * T], s0[:], EXP, scale=SCALE)
                nc.scalar.activation(E1[:, jt * T:(jt + 1) * T], s1[:], EXP, scale=SCALE)

                if jt > 0:
                    av_steps(jt - 1)   # lag exp by one key tile
                if jt in (1, 3) and qk_next:
                    _emit_qk_tile(nc, pools, qk_next.pop(0))
            av_steps(NT - 1)

            normalize(av00, 0, hp, 0)
            normalize(av10, 1, hp, 0)
            normalize(av01, 0, hp, 1)

            # last accumulation (h1, chunk 1) as a short post-pass
            av11 = pp_av.tile([P, 512], F32, tag="av", name="av11")
            for jt in range(NT):
                nc.tensor.matmul(
                    av11[:],
                    vslice(jt, hp, 1),
                    E1[:, jt * T + 512: jt * T + 1024],
                    start=(jt == 0),
                    stop=(jt == NT - 1),
                )
            normalize(av11, 1, hp, 1)

        # ---- output projection ----
        for tt in range(NT):
            po1 = pp_mm.tile([P, 512], F32, tag="mm", name="po1")
            po2 = pp_av.tile([P, 512], F32, tag="av", name="po2")
            for ct in range(NC):
                lhsT = aT[:, ct * T + tt * P: ct * T + tt * P + P]
                nc.tensor.matmul(po1[:], lhsT, wo_sb[:, ct * C: ct * C + 512],
                                 start=(ct == 0), stop=(ct == NC - 1))
            for ct in range(NC):
                lhsT = aT[:, ct * T + tt * P: ct * T + tt * P + P]
                nc.tensor.matmul(po2[:, :256], lhsT, wo_sb[:, ct * C + 512: ct * C + C],
                                 start=(ct == 0), stop=(ct == NC - 1))
            ot = opool.tile([P, C], F32, tag="ot", name="ot")
            nc.scalar.copy(ot[:, 0:512], po1[:])
            nc.scalar.copy(ot[:, 512:C], po2[:, :256])
            nc.sync.dma_start(out_d[tt * P:(tt + 1) * P, :], ot[:])


_CACHED = {}


def build_program():
    if "nc" in _CACHED:
        return _CACHED["nc"]
    nc = bacc.Bacc("TRN2", target_bir_lowering=False, debug=False, num_devices=8)
    x_d = nc.dram_tensor("x", [T, C], F32, kind="ExternalInput").ap()
    wq_d = nc.dram_tensor("w_qkv", [C, 3 * C], F32, kind="ExternalInput").ap()
    wo_d = nc.dram_tensor("w_o", [C, C], F32, kind="ExternalInput").ap()
    out_d = nc.dram_tensor("out", [T, C], F32, kind="ExternalOutput").ap()
    with tile.TileContext(nc) as tc:
        attention_kernel(tc, out_d, x_d, wq_d, wo_d)
    nc.compile()
    _CACHED["nc"] = nc
    return nc


def kernel(x, w_qkv, w_o, _trace=False, _trace_cores=None):
    nc = build_program()
    x = np.ascontiguousarray(np.asarray(x, dtype=np.float32))
    w_qkv = np.ascontiguousarray(np.asarray(w_qkv, dtype=np.float32))
    w_o = np.ascontiguousarray(np.asarray(w_o, dtype=np.float32))
    bs = x.shape[0]
    in_maps = [
        {"x": x[b].reshape(T, C), "w_qkv": w_qkv, "w_o": w_o} for b in range(bs)
    ]
    res = bass_utils.run_bass_kernel_spmd(
        nc, in_maps, core_ids=list(range(bs)), trace=_trace,
        trace_cores=_trace_cores,
    )
    out = np.stack([res.results[b]["out"].reshape(32, 32, C) for b in range(bs)])
    if _trace:
        return out, res
    return out
